# revision 37
# baseline (speedup 1.0000x reference)
"""Trainium2 Bass kernel for 16-head causal self-attention with RoPE.

Problem: x:[2,2048,2048] -> MHA(wq,wk,wv,wo, causal mask, RoPE) -> [2,2048,2048].

Sharding (8 NeuronCores): core = b*4 + g, where b in {0,1} is the batch
(data parallel) and g in {0..3} is a head group of 4 heads (tensor parallel
over the 16 heads / 2048 channels: group g owns channels [g*512, (g+1)*512)).

v2 design (all intermediates SBUF-resident, bf16 matmuls):
  - x and all weights are pre-converted to bf16 on the host and loaded in a
    few large DMAs; q/k (RoPE'd, [dh, S] per head) and v ([S, cw]) live in
    SBUF for the whole kernel - no DRAM roundtrip between phases.
  - The S dimension is processed in 4 chunks of 512; for each chunk qi the
    QKV projections + RoPE (phase A) are immediately followed by the
    attention block for query chunk c=qi (phase B), which by causality only
    needs k/v chunks <= qi.  This interleaving spreads the ScalarE exp work
    and DVE work across the whole runtime so the Tensor engine stays busy.
  - softmax denominator: at-tiles are accumulated on DVE (bf16, 2x mode)
    into one [128,512] tile per (head, chunk); a single ones-matmul then
    reduces it across partitions (instead of one ones-matmul per sk-tile).
  - causal diagonal tiles are widened to >=256 free columns (f32r/bf16
    matmuls below 256 moving columns run at 1/4 rate on the PE); the
    triangular mask is applied in-place on the exp output (bf16 2x DVE).
  - RoPE rotate-half is a [128,128] P-matrix matmul (cross-partition swap
    can't be done on DVE); praw = psum+bias goes through ScalarE, the two
    sin/cos muls run on DVE and the final add runs on the Pool engine.
  - phase C (out = ao @ wo_loc.T) streams psum->SBUF copies through ScalarE
    and DMAs each [128,512] slice out as soon as it lands.
Host: out[b] = sum of the 4 group partials + bo.
"""

import math
import sys

sys.path.insert(0, "/opt/trn_rl_repo")

import numpy as np

N_CORES = 8
B, S, D = 2, 2048, 2048
H, DH = 16, 128
G = 4                 # head groups (tensor-parallel factor per batch)
HPG = H // G          # heads per group = 4
CW = HPG * DH         # channels per group = 512
NT = S // 128         # 16 d-tiles of the contraction dim
SC = 512              # free-dim chunk (one PSUM bank of fp32)
NQ = S // SC          # 4 s-chunks

_NC_CACHE: dict = {}


def build_attn_nc(iters: int = 1, phases: int = 3):
    """Build + compile the Bass module (same program for all 8 cores)."""
    import concourse.tile as tile
    from concourse import bacc, mybir

    f32 = mybir.dt.float32
    bf16 = mybir.dt.bfloat16
    AF = mybir.ActivationFunctionType
    SCALE = 1.0 / math.sqrt(DH)

    nc = bacc.Bacc("TRN2", target_bir_lowering=False, debug=False,
                   num_devices=N_CORES)

    # host-pre-laid-out inputs (see host_prep)
    xTq = nc.dram_tensor("xTq", [NQ, 128, NT, SC], bf16,
                         kind="ExternalInput").ap()
    wqb = nc.dram_tensor("wqb", [128, NT, CW], bf16, kind="ExternalInput").ap()
    wkb = nc.dram_tensor("wkb", [128, NT, CW], bf16, kind="ExternalInput").ap()
    wvb = nc.dram_tensor("wvb", [128, NT, CW], bf16, kind="ExternalInput").ap()
    wob = nc.dram_tensor("wob", [128, HPG, D], bf16, kind="ExternalInput").ap()
    # packed constants (single DMA each): cf32 = bvb | bq | bk,
    # cb16 = PT | ones | mask(zeros|tri)
    cf32d = nc.dram_tensor("cf32d", [128, CW + 2 * HPG], f32,
                           kind="ExternalInput").ap()
    cb16d = nc.dram_tensor("cb16d", [128, 512], bf16,
                           kind="ExternalInput").ap()
    cosq = nc.dram_tensor("cosq", [NQ, DH, SC], bf16,
                          kind="ExternalInput").ap()
    sinq = nc.dram_tensor("sinq", [NQ, DH, SC], bf16,
                          kind="ExternalInput").ap()

    out = nc.dram_tensor("out", [S, D], bf16, kind="ExternalOutput").ap()

    with tile.TileContext(nc) as tc:
        for it in range(iters):
            with tc.tile_pool(name="const", bufs=1) as cpool, \
                 tc.tile_pool(name="wts", bufs=1) as wpool, \
                 tc.tile_pool(name="perst", bufs=1) as ppool:
                # ---- persistent SBUF tensors --------------------------
                qT = [ppool.tile([DH, S], bf16, name=f"qT{h}_{it}",
                                 tag=f"qT{h}") for h in range(HPG)]
                kT = [ppool.tile([DH, S], bf16, name=f"kT{h}_{it}",
                                 tag=f"kT{h}") for h in range(HPG)]
                v_t = [ppool.tile([128, CW], bf16, name=f"v{t}_{it}",
                                  tag=f"v{t}") for t in range(NT)]
                aoT = ppool.tile([128, HPG * S], bf16, name=f"aoT_{it}",
                                 tag="aoT")

                # ---- startup DMAs (order = queue service order) -------
                # scalar queue: wq parts first so the first matmul can start
                # ~2us in, then the small consts, then wk/wv/wo.
                # sync queue: x chunk 0 + cos/sin.
                w_sb = {}
                for nm, dram in (("q", wqb),):
                    t = wpool.tile([128, NT, CW], bf16, name=f"w{nm}_{it}",
                                   tag=f"w{nm}")
                    for j0, j1 in ((0, 1), (1, 2), (2, 4), (4, 8), (8, 16)):
                        nc.scalar.dma_start(t[:, j0:j1, :],
                                            dram[:, j0:j1, :])
                    w_sb[nm] = t
                cb16 = cpool.tile([128, 512], bf16, name=f"cb16{it}",
                                  tag="cb16")
                nc.scalar.dma_start(cb16[:], cb16d[:])
                cf32 = cpool.tile([128, CW + 2 * HPG], f32,
                                  name=f"cf32{it}", tag="cf32")
                nc.scalar.dma_start(cf32[:], cf32d[:])
                pt_sb = cb16[:, 0:DH]
                ones_sb = cb16[:, 128:256]
                msk_full = cb16[:, 256:512]   # [zeros(128) | tri(128)]
                msk_tri = cb16[:, 384:512]
                bvb_sb = cf32[:, 0:CW]
                bq_sb = [cf32[:, CW + ct:CW + ct + 1] for ct in range(HPG)]
                bk_sb = [cf32[:, CW + HPG + ct:CW + HPG + ct + 1]
                         for ct in range(HPG)]
                # wk/wv/wo arrive during compute: one large DMA each
                # minimizes shared-pipe overhead
                for nm, dram in (("k", wkb), ("v", wvb)):
                    t = wpool.tile([128, NT, CW], bf16, name=f"w{nm}_{it}",
                                   tag=f"w{nm}")
                    nc.scalar.dma_start(t[:], dram[:])
                    w_sb[nm] = t
                wo_sb = wpool.tile([128, HPG, D], bf16, name=f"wo{it}",
                                   tag="wo")
                nc.scalar.dma_start(wo_sb[:], wob[:])

                with tc.tile_pool(name="xqp", bufs=2) as xqpool, \
                     tc.tile_pool(name="csp", bufs=2) as cspool, \
                     tc.tile_pool(name="prawp", bufs=2) as prawp, \
                     tc.tile_pool(name="workA", bufs=2) as wkp, \
                     tc.tile_pool(name="atp", bufs=6) as atpool, \
                     tc.tile_pool(name="accp", bufs=2) as accpool, \
                     tc.tile_pool(name="recp", bufs=2) as recpool, \
                     tc.tile_pool(name="outp", bufs=6) as outpool, \
                     tc.tile_pool(name="psA", bufs=3, space="PSUM") as psA, \
                     tc.tile_pool(name="psT", bufs=3, space="PSUM") as psT, \
                     tc.tile_pool(name="psO", bufs=2, space="PSUM") as psO:

                    def load_chunk(qi):
                        xq = xqpool.tile([128, NT, SC], bf16,
                                         name=f"xq{qi}_{it}", tag="xq")
                        cos_c = cspool.tile([DH, SC], bf16,
                                            name=f"cos{qi}_{it}", tag="cos")
                        sin_c = cspool.tile([DH, SC], bf16,
                                            name=f"sin{qi}_{it}", tag="sin")
                        if qi == 0:
                            # fine ramped parts: first matmul starts early
                            for j0, j1 in ((0, 1), (1, 2), (2, 4), (4, 8),
                                           (8, 16)):
                                nc.sync.dma_start(xq[:, j0:j1, :],
                                                  xTq[qi][:, j0:j1, :])
                        else:
                            nc.sync.dma_start(xq[:], xTq[qi])
                        nc.sync.dma_start(cos_c[:], cosq[qi])
                        nc.sync.dma_start(sin_c[:], sinq[qi])
                        return xq, cos_c, sin_c

                    def c_block_st(st, in_tail=False):
                        # one out-projection s-tile; interleaved between
                        # attention head-blocks so its matmuls fill PE gaps
                        # left by the exp-bound B-blocks.
                        for dcp in range(0, 4, 2):
                            psa = psA.tile([128, SC], f32,
                                           name=f"op{st}{dcp}_{it}",
                                           tag="ps")
                            psb = psO.tile([128, SC], f32,
                                           name=f"op{st}{dcp+1}_{it}",
                                           tag="oT")
                            for h in range(HPG):
                                lhs = aoT[:, h * S + st * 128:
                                          h * S + (st + 1) * 128]
                                nc.tensor.matmul(
                                    psa[:], lhs,
                                    wo_sb[:, h, dcp * SC:(dcp + 1) * SC],
                                    start=(h == 0), stop=(h == HPG - 1))
                                nc.tensor.matmul(
                                    psb[:], lhs,
                                    wo_sb[:, h,
                                          (dcp + 1) * SC:(dcp + 2) * SC],
                                    start=(h == 0), stop=(h == HPG - 1))
                            for dc, op in ((dcp, psa), (dcp + 1, psb)):
                                ot = outpool.tile([128, SC], bf16,
                                                  name=f"ot{st}{dc}_{it}",
                                                  tag="ot")
                                if dc % 2 == 0:
                                    nc.scalar.activation(ot[:], op[:],
                                                         AF.Copy)
                                else:
                                    nc.vector.tensor_copy(ot[:], op[:])
                                nc.sync.dma_start(
                                    out[st * 128:(st + 1) * 128,
                                        dc * SC:(dc + 1) * SC], ot[:])

                    C_FILL = {(1, 0): [0], (1, 1): [1],
                              (2, 0): [2], (2, 1): [3], (2, 2): [4],
                              (3, 0): [5, 6], (3, 1): [7, 8],
                              (3, 2): [9, 10], (3, 3): [11]}
                    xq, cos_c, sin_c = load_chunk(0)
                    for qi in range(NQ):
                        # ---- phase A: q/k projections + RoPE ----------
                        for nm, bias_sb, qkT in (("q", bq_sb, qT),
                                                 ("k", bk_sb, kT)):
                            for cp in range(0, HPG, 2):
                                psa = psA.tile([128, SC], f32,
                                               name=f"ps{nm}{cp}_{qi}_{it}",
                                               tag="ps")
                                psb = psA.tile([128, SC], f32,
                                               name=f"ps{nm}{cp+1}_{qi}_{it}",
                                               tag="ps")
                                for d in range(NT):
                                    nc.tensor.matmul(
                                        psa[:],
                                        w_sb[nm][:, d, cp * DH:(cp + 1) * DH],
                                        xq[:, d, :],
                                        start=(d == 0), stop=(d == NT - 1))
                                    nc.tensor.matmul(
                                        psb[:],
                                        w_sb[nm][:, d,
                                                 (cp + 1) * DH:(cp + 2) * DH],
                                        xq[:, d, :],
                                        start=(d == 0), stop=(d == NT - 1))
                                for ct, ps in ((cp, psa), (cp + 1, psb)):
                                    praw = prawp.tile(
                                        [128, SC], bf16,
                                        name=f"praw{nm}{ct}_{qi}_{it}",
                                        tag=f"praw{ct}")
                                    nc.vector.tensor_scalar_add(
                                        praw[:], ps[:], bias_sb[ct])
                                    psr = psT.tile([128, SC], f32,
                                                   name=f"psr{nm}{ct}_{qi}_{it}",
                                                   tag="tr")
                                    nc.tensor.matmul(psr[:], pt_sb,
                                                     praw[:],
                                                     start=True, stop=True)
                                    m1 = wkp.tile([128, SC], bf16,
                                                  name=f"m1{nm}{ct}_{qi}_{it}",
                                                  tag="m1")
                                    nc.vector.tensor_mul(m1[:], praw[:],
                                                         cos_c[:])
                                    m2 = wkp.tile([128, SC], bf16,
                                                  name=f"m2{nm}{ct}_{qi}_{it}",
                                                  tag="m2")
                                    nc.vector.tensor_mul(m2[:], psr[:],
                                                         sin_c[:])
                                    nc.gpsimd.tensor_add(
                                        qkT[ct][:, qi * SC:(qi + 1) * SC],
                                        m1[:], m2[:])

                        # ---- phase A: v projection --------------------
                        for sp in range(0, 4, 2):
                            psa = psA.tile([128, SC], f32,
                                           name=f"psv{sp}_{qi}_{it}",
                                           tag="ps")
                            psb = psA.tile([128, SC], f32,
                                           name=f"psv{sp+1}_{qi}_{it}",
                                           tag="ps")
                            for d in range(NT):
                                nc.tensor.matmul(
                                    psa[:],
                                    xq[:, d, sp * 128:(sp + 1) * 128],
                                    w_sb["v"][:, d, :],
                                    start=(d == 0), stop=(d == NT - 1))
                                nc.tensor.matmul(
                                    psb[:],
                                    xq[:, d, (sp + 1) * 128:(sp + 2) * 128],
                                    w_sb["v"][:, d, :],
                                    start=(d == 0), stop=(d == NT - 1))
                            for st, ps in ((sp, psa), (sp + 1, psb)):
                                nc.vector.tensor_add(v_t[qi * 4 + st][:],
                                                     ps[:], bvb_sb)

                        # prefetch next x chunk before this block's out DMAs
                        if qi + 1 < NQ:
                            nxt = load_chunk(qi + 1)

                        # ---- phase B: attention for query chunk c=qi --
                        c = qi
                        ntile = 4 * c + 4
                        for h in range(HPG):
                            oT = psO.tile([DH, SC], f32,
                                          name=f"oT{h}{c}_{it}", tag="oT")
                            # acc2 holds a PAIR of at tiles; the first pair's
                            # exps write it directly, later pairs need only
                            # ONE [128,1024] DVE add each (fewer DVE ops:
                            # per-op overhead is the scarce resource on hw)
                            acc2 = accpool.tile([128, 2, SC], bf16,
                                                name=f"acc{h}{c}_{it}",
                                                tag="acc")

                            def b_tile(t_, at, k):
                                rr = t_ - 4 * c
                                n0 = max(rr, 0) * 128
                                sps = psT.tile([128, SC], f32,
                                               name=f"sps{h}{c}{t_}_{it}",
                                               tag="tr")
                                nc.tensor.matmul(
                                    sps[:, n0:],
                                    kT[h][:, t_ * 128:(t_ + 1) * 128],
                                    qT[h][:, c * SC + n0:(c + 1) * SC],
                                    start=True, stop=True)
                                nc.scalar.activation(at[:, n0:], sps[:, n0:],
                                                     AF.Exp, bias=0.0,
                                                     scale=SCALE)
                                if rr >= 0:
                                    blk = slice(rr * 128, (rr + 1) * 128)
                                    nc.vector.tensor_mul(
                                        at[:, blk], at[:, blk], msk_tri)
                                nc.tensor.matmul(
                                    oT[:, n0:],
                                    v_t[t_][:, h * DH:(h + 1) * DH],
                                    at[:, n0:],
                                    start=(t_ == 0), stop=(t_ == ntile - 1),
                                    skip_group_check=True)
                                return n0

                            # full-tile pairs (none for c == 0)
                            for tp in range(2 * c):
                                t0_, t1_ = 2 * tp, 2 * tp + 1
                                if tp == 0:
                                    at2 = acc2
                                else:
                                    at2 = atpool.tile(
                                        [128, 2, SC], bf16,
                                        name=f"at{h}{c}{tp}_{it}", tag="at")
                                b_tile(t0_, at2[:, 0, :], 0)
                                b_tile(t1_, at2[:, 1, :], 1)
                                if tp > 0:
                                    a2f = acc2[:].rearrange(
                                        "p a b -> p (a b)")
                                    nc.vector.tensor_add(
                                        a2f, a2f,
                                        at2[:].rearrange("p a b -> p (a b)"))
                            # diagonal tiles (single, masked); for c == 0
                            # only half 0 of acc2 is used (half 1 would be
                            # uninitialized garbage in the denominator)
                            for t_ in range(4 * c, ntile):
                                if c == 0 and t_ == 0:
                                    at = acc2[:, 0, :]
                                else:
                                    at = atpool.tile(
                                        [128, 2, SC], bf16,
                                        name=f"atd{h}{c}{t_}_{it}",
                                        tag="at")[:, 0, :]
                                n0 = b_tile(t_, at, 0)
                                if not (c == 0 and t_ == 0):
                                    nc.vector.tensor_add(
                                        acc2[:, 0, n0:], acc2[:, 0, n0:],
                                        at[:, n0:])
                            dnp = psT.tile([128, SC], f32,
                                           name=f"dn{h}{c}_{it}", tag="tr")
                            nc.tensor.matmul(dnp[:], ones_sb, acc2[:, 0, :],
                                             start=True, stop=(c == 0))
                            if c > 0:
                                nc.tensor.matmul(dnp[:], ones_sb,
                                                 acc2[:, 1, :],
                                                 start=False, stop=True,
                                                 skip_group_check=True)
                            rec = recpool.tile([128, SC], f32,
                                               name=f"rec{h}{c}_{it}",
                                               tag="rec")
                            nc.vector.reciprocal(rec[:], dnp[:])
                            nc.vector.tensor_mul(
                                aoT[:, h * S + c * SC:h * S + (c + 1) * SC],
                                oT[:], rec[:])
                            # earlier chunks' out-projection tiles as PE
                            # gap fillers, weighted toward the exp-heavy
                            # late chunks
                            for st in C_FILL.get((qi, h), ()):
                                c_block_st(st)
                        if qi + 1 < NQ:
                            xq, cos_c, sin_c = nxt

                    # ---- tail: out-projection for the last chunk ------
                    for st in range(4 * (NQ - 1), 4 * NQ):
                        c_block_st(st, in_tail=True)
    nc.compile()
    return nc


def host_prep(inputs: dict) -> list:
    """Build per-core input maps (host-side sharding + bf16 relayout)."""
    import ml_dtypes
    bf16 = ml_dtypes.bfloat16

    x = np.asarray(inputs["x"], dtype=np.float32)
    wq = np.asarray(inputs["wq"], dtype=np.float32)
    wk = np.asarray(inputs["wk"], dtype=np.float32)
    wv = np.asarray(inputs["wv"], dtype=np.float32)
    wo = np.asarray(inputs["wo"], dtype=np.float32)
    bq = np.asarray(inputs["bq"], dtype=np.float32)
    bk = np.asarray(inputs["bk"], dtype=np.float32)
    bv = np.asarray(inputs["bv"], dtype=np.float32)

    inv = 1.0 / (10000.0 ** (np.arange(0, DH, 2, dtype=np.float64) / DH))
    ang = np.arange(S, dtype=np.float64)[:, None] * inv[None, :]
    sin = np.repeat(np.sin(ang), 2, axis=1).astype(np.float32)  # [S, DH]
    cos = np.repeat(np.cos(ang), 2, axis=1).astype(np.float32)
    # [NQ, DH, SC]: cosq[qi, p, s] = cos[qi*SC+s, p]
    cosq = np.ascontiguousarray(
        cos.reshape(NQ, SC, DH).transpose(0, 2, 1)).astype(bf16)
    sinq = np.ascontiguousarray(
        sin.reshape(NQ, SC, DH).transpose(0, 2, 1)).astype(bf16)

    P = np.zeros((DH, DH), np.float32)
    idx = np.arange(0, DH, 2)
    P[idx, idx + 1] = -1.0    # out[2i]   = -x[2i+1]
    P[idx + 1, idx] = 1.0     # out[2i+1] =  x[2i]
    PT = np.ascontiguousarray(P.T)

    # packed bf16 consts: PT | ones | zeros | tri(keep jj >= i)
    tri = (np.arange(128)[None, :] >= np.arange(128)[:, None])
    cb16 = np.concatenate(
        [PT, np.ones((128, 128), np.float32),
         np.zeros((128, 128), np.float32), tri.astype(np.float32)],
        axis=1).astype(bf16)

    # [NQ, 128, NT, SC]: xTq[qi, p, d, s] = x[b][qi*SC+s, d*128+p]
    xTqb = [np.ascontiguousarray(
        x[b].reshape(NQ, SC, NT, 128).transpose(0, 3, 2, 1)).astype(bf16)
        for b in range(B)]

    in_maps = []
    for core in range(N_CORES):
        b, g = divmod(core, G)
        c0 = g * CW
        # [128, NT, CW]: wqb[p, d, c] = wq[c0+c, d*128+p]
        wqb = np.ascontiguousarray(
            wq[c0:c0 + CW, :].reshape(CW, NT, 128).transpose(2, 1, 0)
        ).astype(bf16)
        wkb = np.ascontiguousarray(
            wk[c0:c0 + CW, :].reshape(CW, NT, 128).transpose(2, 1, 0)
        ).astype(bf16)
        wvb = np.ascontiguousarray(
            wv[c0:c0 + CW, :].reshape(CW, NT, 128).transpose(2, 1, 0)
        ).astype(bf16)
        # [128, HPG, D]: wob[p, h, j] = wo[j, c0+h*128+p]
        wob = np.ascontiguousarray(
            wo[:, c0:c0 + CW].reshape(D, HPG, 128).transpose(2, 1, 0)
        ).astype(bf16)
        # packed f32 consts: bvb (broadcast) | bq columns | bk columns
        cf32 = np.zeros((128, CW + 2 * HPG), np.float32)
        cf32[:, 0:CW] = bv[c0:c0 + CW][None, :]
        cf32[:, CW:CW + HPG] = bq[c0:c0 + CW].reshape(HPG, DH).T
        cf32[:, CW + HPG:] = bk[c0:c0 + CW].reshape(HPG, DH).T
        in_maps.append({
            "xTq": xTqb[b],
            "wqb": wqb,
            "wkb": wkb,
            "wvb": wvb,
            "wob": wob,
            "cf32d": cf32,
            "cb16d": cb16,
            "cosq": cosq,
            "sinq": sinq,
        })
    return in_maps


def _get_nc():
    if "nc" not in _NC_CACHE:
        _NC_CACHE["nc"] = build_attn_nc(iters=1)
    return _NC_CACHE["nc"]


def kernel(**inputs) -> np.ndarray:
    from concourse.bass_utils import run_bass_kernel_spmd

    nc = _get_nc()
    in_maps = host_prep(inputs)
    res = run_bass_kernel_spmd(nc, in_maps, core_ids=list(range(N_CORES)))
    bo = np.asarray(inputs["bo"], dtype=np.float32)
    outp = np.zeros((B, S, D), np.float32)
    for core in range(N_CORES):
        outp[core // G] += np.asarray(res.results[core]["out"],
                                      dtype=np.float32)
    outp += bo[None, None, :]
    return outp


# revision 38
# speedup vs baseline: 1.0059x; 1.0059x over previous
"""Trainium2 Bass kernel for 16-head causal self-attention with RoPE.

Problem: x:[2,2048,2048] -> MHA(wq,wk,wv,wo, causal mask, RoPE) -> [2,2048,2048].

Sharding (8 NeuronCores): core = b*4 + g, where b in {0,1} is the batch
(data parallel) and g in {0..3} is a head group of 4 heads (tensor parallel
over the 16 heads / 2048 channels: group g owns channels [g*512, (g+1)*512)).

v2 design (all intermediates SBUF-resident, bf16 matmuls):
  - x and all weights are pre-converted to bf16 on the host and loaded in a
    few large DMAs; q/k (RoPE'd, [dh, S] per head) and v ([S, cw]) live in
    SBUF for the whole kernel - no DRAM roundtrip between phases.
  - The S dimension is processed in 4 chunks of 512; for each chunk qi the
    QKV projections + RoPE (phase A) are immediately followed by the
    attention block for query chunk c=qi (phase B), which by causality only
    needs k/v chunks <= qi.  This interleaving spreads the ScalarE exp work
    and DVE work across the whole runtime so the Tensor engine stays busy.
  - softmax denominator: at-tiles are accumulated on DVE (bf16, 2x mode)
    into one [128,512] tile per (head, chunk); a single ones-matmul then
    reduces it across partitions (instead of one ones-matmul per sk-tile).
  - causal diagonal tiles are widened to >=256 free columns (f32r/bf16
    matmuls below 256 moving columns run at 1/4 rate on the PE); the
    triangular mask is applied in-place on the exp output (bf16 2x DVE).
  - RoPE rotate-half is a [128,128] P-matrix matmul (cross-partition swap
    can't be done on DVE); praw = psum+bias goes through ScalarE, the two
    sin/cos muls run on DVE and the final add runs on the Pool engine.
  - phase C (out = ao @ wo_loc.T) streams psum->SBUF copies through ScalarE
    and DMAs each [128,512] slice out as soon as it lands.
Host: out[b] = sum of the 4 group partials + bo.
"""

import math
import sys

sys.path.insert(0, "/opt/trn_rl_repo")

import numpy as np

N_CORES = 8
B, S, D = 2, 2048, 2048
H, DH = 16, 128
G = 4                 # head groups (tensor-parallel factor per batch)
HPG = H // G          # heads per group = 4
CW = HPG * DH         # channels per group = 512
NT = S // 128         # 16 d-tiles of the contraction dim
SC = 512              # free-dim chunk (one PSUM bank of fp32)
NQ = S // SC          # 4 s-chunks

_NC_CACHE: dict = {}


def build_attn_nc(iters: int = 1, phases: int = 3):
    """Build + compile the Bass module (same program for all 8 cores)."""
    import concourse.tile as tile
    from concourse import bacc, mybir

    f32 = mybir.dt.float32
    bf16 = mybir.dt.bfloat16
    AF = mybir.ActivationFunctionType
    SCALE = 1.0 / math.sqrt(DH)

    nc = bacc.Bacc("TRN2", target_bir_lowering=False, debug=False,
                   num_devices=N_CORES)

    # host-pre-laid-out inputs (see host_prep)
    xTq = nc.dram_tensor("xTq", [NQ, 128, NT, SC], bf16,
                         kind="ExternalInput").ap()
    wqb = nc.dram_tensor("wqb", [128, NT, CW], bf16, kind="ExternalInput").ap()
    wkb = nc.dram_tensor("wkb", [128, NT, CW], bf16, kind="ExternalInput").ap()
    wvb = nc.dram_tensor("wvb", [128, NT, CW], bf16, kind="ExternalInput").ap()
    wob = nc.dram_tensor("wob", [128, HPG, D], bf16, kind="ExternalInput").ap()
    # packed constants (single DMA each): cf32 = bvb | bq | bk,
    # cb16 = PT | ones | mask(zeros|tri)
    cf32d = nc.dram_tensor("cf32d", [128, CW + 2 * HPG], f32,
                           kind="ExternalInput").ap()
    cb16d = nc.dram_tensor("cb16d", [128, 512], bf16,
                           kind="ExternalInput").ap()
    cosq = nc.dram_tensor("cosq", [NQ, DH, SC], bf16,
                          kind="ExternalInput").ap()
    sinq = nc.dram_tensor("sinq", [NQ, DH, SC], bf16,
                          kind="ExternalInput").ap()

    out = nc.dram_tensor("out", [S, D], bf16, kind="ExternalOutput").ap()

    with tile.TileContext(nc) as tc:
        for it in range(iters):
            with tc.tile_pool(name="const", bufs=1) as cpool, \
                 tc.tile_pool(name="wts", bufs=1) as wpool, \
                 tc.tile_pool(name="perst", bufs=1) as ppool:
                # ---- persistent SBUF tensors --------------------------
                qT = [ppool.tile([DH, S], bf16, name=f"qT{h}_{it}",
                                 tag=f"qT{h}") for h in range(HPG)]
                kT = [ppool.tile([DH, S], bf16, name=f"kT{h}_{it}",
                                 tag=f"kT{h}") for h in range(HPG)]
                v_t = [ppool.tile([128, CW], bf16, name=f"v{t}_{it}",
                                  tag=f"v{t}") for t in range(NT)]
                aoT = ppool.tile([128, HPG * S], bf16, name=f"aoT_{it}",
                                 tag="aoT")

                # ---- startup DMAs (order = queue service order) -------
                # scalar queue: wq parts first so the first matmul can start
                # ~2us in, then the small consts, then wk/wv/wo.
                # sync queue: x chunk 0 + cos/sin.
                w_sb = {}
                for nm, dram in (("q", wqb),):
                    t = wpool.tile([128, NT, CW], bf16, name=f"w{nm}_{it}",
                                   tag=f"w{nm}")
                    for j0, j1 in ((0, 1), (1, 2), (2, 4), (4, 8), (8, 16)):
                        nc.scalar.dma_start(t[:, j0:j1, :],
                                            dram[:, j0:j1, :])
                    w_sb[nm] = t
                cb16 = cpool.tile([128, 512], bf16, name=f"cb16{it}",
                                  tag="cb16")
                nc.scalar.dma_start(cb16[:], cb16d[:])
                cf32 = cpool.tile([128, CW + 2 * HPG], f32,
                                  name=f"cf32{it}", tag="cf32")
                nc.scalar.dma_start(cf32[:], cf32d[:])
                pt_sb = cb16[:, 0:DH]
                ones_sb = cb16[:, 128:256]
                msk_full = cb16[:, 256:512]   # [zeros(128) | tri(128)]
                msk_tri = cb16[:, 384:512]
                bvb_sb = cf32[:, 0:CW]
                bq_sb = [cf32[:, CW + ct:CW + ct + 1] for ct in range(HPG)]
                bk_sb = [cf32[:, CW + HPG + ct:CW + HPG + ct + 1]
                         for ct in range(HPG)]
                # wk/wv/wo arrive during compute: one large DMA each
                # minimizes shared-pipe overhead
                for nm, dram in (("k", wkb), ("v", wvb)):
                    t = wpool.tile([128, NT, CW], bf16, name=f"w{nm}_{it}",
                                   tag=f"w{nm}")
                    nc.scalar.dma_start(t[:], dram[:])
                    w_sb[nm] = t
                wo_sb = wpool.tile([128, HPG, D], bf16, name=f"wo{it}",
                                   tag="wo")
                nc.scalar.dma_start(wo_sb[:], wob[:])

                with tc.tile_pool(name="xqp", bufs=2) as xqpool, \
                     tc.tile_pool(name="csp", bufs=2) as cspool, \
                     tc.tile_pool(name="prawp", bufs=2) as prawp, \
                     tc.tile_pool(name="workA", bufs=2) as wkp, \
                     tc.tile_pool(name="atp", bufs=6) as atpool, \
                     tc.tile_pool(name="accp", bufs=2) as accpool, \
                     tc.tile_pool(name="recp", bufs=2) as recpool, \
                     tc.tile_pool(name="outp", bufs=6) as outpool, \
                     tc.tile_pool(name="psA", bufs=3, space="PSUM") as psA, \
                     tc.tile_pool(name="psT", bufs=3, space="PSUM") as psT, \
                     tc.tile_pool(name="psO", bufs=2, space="PSUM") as psO:

                    def load_chunk(qi):
                        xq = xqpool.tile([128, NT, SC], bf16,
                                         name=f"xq{qi}_{it}", tag="xq")
                        cos_c = cspool.tile([DH, SC], bf16,
                                            name=f"cos{qi}_{it}", tag="cos")
                        sin_c = cspool.tile([DH, SC], bf16,
                                            name=f"sin{qi}_{it}", tag="sin")
                        if qi == 0:
                            # fine ramped parts: first matmul starts early
                            for j0, j1 in ((0, 1), (1, 2), (2, 4), (4, 8),
                                           (8, 16)):
                                nc.sync.dma_start(xq[:, j0:j1, :],
                                                  xTq[qi][:, j0:j1, :])
                        else:
                            nc.sync.dma_start(xq[:], xTq[qi])
                        nc.sync.dma_start(cos_c[:], cosq[qi])
                        nc.sync.dma_start(sin_c[:], sinq[qi])
                        return xq, cos_c, sin_c

                    def c_block_st(st):
                        # one out-projection s-tile; interleaved between
                        # attention head-blocks so its matmuls fill PE gaps
                        # left by the exp-bound B-blocks.
                        for dcp in range(0, 4, 2):
                            psa = psA.tile([128, SC], f32,
                                           name=f"op{st}{dcp}_{it}",
                                           tag="ps")
                            psb = psO.tile([128, SC], f32,
                                           name=f"op{st}{dcp+1}_{it}",
                                           tag="oT")
                            for h in range(HPG):
                                lhs = aoT[:, h * S + st * 128:
                                          h * S + (st + 1) * 128]
                                nc.tensor.matmul(
                                    psa[:], lhs,
                                    wo_sb[:, h, dcp * SC:(dcp + 1) * SC],
                                    start=(h == 0), stop=(h == HPG - 1))
                                nc.tensor.matmul(
                                    psb[:], lhs,
                                    wo_sb[:, h,
                                          (dcp + 1) * SC:(dcp + 2) * SC],
                                    start=(h == 0), stop=(h == HPG - 1))
                            for dc, op in ((dcp, psa), (dcp + 1, psb)):
                                ot = outpool.tile([128, SC], bf16,
                                                  name=f"ot{st}{dc}_{it}",
                                                  tag="ot")
                                if dc % 2 == 0:
                                    nc.scalar.activation(ot[:], op[:],
                                                         AF.Copy)
                                else:
                                    nc.vector.tensor_copy(ot[:], op[:])
                                nc.sync.dma_start(
                                    out[st * 128:(st + 1) * 128,
                                        dc * SC:(dc + 1) * SC], ot[:])

                    C_FILL = {(1, 0): [0], (1, 1): [1],
                              (2, 0): [2], (2, 1): [3], (2, 2): [4],
                              (3, 0): [5, 6], (3, 1): [7, 8],
                              (3, 2): [9, 10], (3, 3): [11]}
                    xq, cos_c, sin_c = load_chunk(0)
                    for qi in range(NQ):
                        # ---- phase A: q/k projections + RoPE ----------
                        for nm, bias_sb, qkT in (("q", bq_sb, qT),
                                                 ("k", bk_sb, kT)):
                            for cp in range(0, HPG, 2):
                                psa = psA.tile([128, SC], f32,
                                               name=f"ps{nm}{cp}_{qi}_{it}",
                                               tag="ps")
                                psb = psA.tile([128, SC], f32,
                                               name=f"ps{nm}{cp+1}_{qi}_{it}",
                                               tag="ps")
                                for d in range(NT):
                                    nc.tensor.matmul(
                                        psa[:],
                                        w_sb[nm][:, d, cp * DH:(cp + 1) * DH],
                                        xq[:, d, :],
                                        start=(d == 0), stop=(d == NT - 1))
                                    nc.tensor.matmul(
                                        psb[:],
                                        w_sb[nm][:, d,
                                                 (cp + 1) * DH:(cp + 2) * DH],
                                        xq[:, d, :],
                                        start=(d == 0), stop=(d == NT - 1))
                                for ct, ps in ((cp, psa), (cp + 1, psb)):
                                    praw = prawp.tile(
                                        [128, SC], bf16,
                                        name=f"praw{nm}{ct}_{qi}_{it}",
                                        tag=f"praw{ct}")
                                    nc.vector.tensor_scalar_add(
                                        praw[:], ps[:], bias_sb[ct])
                                    psr = psT.tile([128, SC], f32,
                                                   name=f"psr{nm}{ct}_{qi}_{it}",
                                                   tag="tr")
                                    nc.tensor.matmul(psr[:], pt_sb,
                                                     praw[:],
                                                     start=True, stop=True)
                                    m1 = wkp.tile([128, SC], bf16,
                                                  name=f"m1{nm}{ct}_{qi}_{it}",
                                                  tag="m1")
                                    nc.vector.tensor_mul(m1[:], praw[:],
                                                         cos_c[:])
                                    m2 = wkp.tile([128, SC], bf16,
                                                  name=f"m2{nm}{ct}_{qi}_{it}",
                                                  tag="m2")
                                    nc.vector.tensor_mul(m2[:], psr[:],
                                                         sin_c[:])
                                    nc.gpsimd.tensor_add(
                                        qkT[ct][:, qi * SC:(qi + 1) * SC],
                                        m1[:], m2[:])

                        # ---- phase A: v projection --------------------
                        for sp in range(0, 4, 2):
                            psa = psA.tile([128, SC], f32,
                                           name=f"psv{sp}_{qi}_{it}",
                                           tag="ps")
                            psb = psA.tile([128, SC], f32,
                                           name=f"psv{sp+1}_{qi}_{it}",
                                           tag="ps")
                            for d in range(NT):
                                nc.tensor.matmul(
                                    psa[:],
                                    xq[:, d, sp * 128:(sp + 1) * 128],
                                    w_sb["v"][:, d, :],
                                    start=(d == 0), stop=(d == NT - 1))
                                nc.tensor.matmul(
                                    psb[:],
                                    xq[:, d, (sp + 1) * 128:(sp + 2) * 128],
                                    w_sb["v"][:, d, :],
                                    start=(d == 0), stop=(d == NT - 1))
                            for st, ps in ((sp, psa), (sp + 1, psb)):
                                nc.vector.tensor_add(v_t[qi * 4 + st][:],
                                                     ps[:], bvb_sb)

                        # prefetch next x chunk before this block's out DMAs
                        if qi + 1 < NQ:
                            nxt = load_chunk(qi + 1)

                        # ---- phase B: attention for query chunk c=qi --
                        c = qi
                        ntile = 4 * c + 4
                        for h in range(HPG):
                            oT = psO.tile([DH, SC], f32,
                                          name=f"oT{h}{c}_{it}", tag="oT")
                            acc = accpool.tile([128, SC], bf16,
                                               name=f"acc{h}{c}_{it}",
                                               tag="acc")
                            for t_ in range(ntile):
                                rr = t_ - 4 * c
                                n0 = max(rr, 0) * 128
                                sps = psT.tile([128, SC], f32,
                                               name=f"sps{h}{c}{t_}_{it}",
                                               tag="tr")
                                nc.tensor.matmul(
                                    sps[:, n0:],
                                    kT[h][:, t_ * 128:(t_ + 1) * 128],
                                    qT[h][:, c * SC + n0:(c + 1) * SC],
                                    start=True, stop=True)
                                # tile 0's exp goes straight into the
                                # denominator accumulator (saves a copy);
                                # later tiles' adds WAR-wait on its PV read
                                if t_ == 0:
                                    at = acc
                                else:
                                    at = atpool.tile([128, SC], bf16,
                                                     name=f"at{h}{c}{t_}_{it}",
                                                     tag="at")
                                nc.scalar.activation(at[:, n0:], sps[:, n0:],
                                                     AF.Exp, bias=0.0,
                                                     scale=SCALE)
                                if rr >= 0:
                                    blk = slice(rr * 128, (rr + 1) * 128)
                                    nc.vector.tensor_mul(
                                        at[:, blk], at[:, blk], msk_tri)
                                nc.tensor.matmul(
                                    oT[:, n0:],
                                    v_t[t_][:, h * DH:(h + 1) * DH],
                                    at[:, n0:],
                                    start=(t_ == 0), stop=(t_ == ntile - 1),
                                    skip_group_check=True)
                                if t_ > 0:
                                    nc.vector.tensor_add(acc[:, n0:],
                                                         acc[:, n0:],
                                                         at[:, n0:])
                            dnp = psT.tile([128, SC], f32,
                                           name=f"dn{h}{c}_{it}", tag="tr")
                            nc.tensor.matmul(dnp[:], ones_sb, acc[:],
                                             start=True, stop=True)
                            rec = recpool.tile([128, SC], f32,
                                               name=f"rec{h}{c}_{it}",
                                               tag="rec")
                            nc.vector.reciprocal(rec[:], dnp[:])
                            nc.vector.tensor_mul(
                                aoT[:, h * S + c * SC:h * S + (c + 1) * SC],
                                oT[:], rec[:])
                            # earlier chunks' out-projection tiles as PE
                            # gap fillers, weighted toward the exp-heavy
                            # late chunks
                            for st in C_FILL.get((qi, h), ()):
                                c_block_st(st)
                        if qi + 1 < NQ:
                            xq, cos_c, sin_c = nxt

                    # ---- tail: out-projection for the last chunk ------
                    for st in range(4 * (NQ - 1), 4 * NQ):
                        c_block_st(st)
    nc.compile()
    return nc


def host_prep(inputs: dict) -> list:
    """Build per-core input maps (host-side sharding + bf16 relayout)."""
    import ml_dtypes
    bf16 = ml_dtypes.bfloat16

    x = np.asarray(inputs["x"], dtype=np.float32)
    wq = np.asarray(inputs["wq"], dtype=np.float32)
    wk = np.asarray(inputs["wk"], dtype=np.float32)
    wv = np.asarray(inputs["wv"], dtype=np.float32)
    wo = np.asarray(inputs["wo"], dtype=np.float32)
    bq = np.asarray(inputs["bq"], dtype=np.float32)
    bk = np.asarray(inputs["bk"], dtype=np.float32)
    bv = np.asarray(inputs["bv"], dtype=np.float32)

    inv = 1.0 / (10000.0 ** (np.arange(0, DH, 2, dtype=np.float64) / DH))
    ang = np.arange(S, dtype=np.float64)[:, None] * inv[None, :]
    sin = np.repeat(np.sin(ang), 2, axis=1).astype(np.float32)  # [S, DH]
    cos = np.repeat(np.cos(ang), 2, axis=1).astype(np.float32)
    # [NQ, DH, SC]: cosq[qi, p, s] = cos[qi*SC+s, p]
    cosq = np.ascontiguousarray(
        cos.reshape(NQ, SC, DH).transpose(0, 2, 1)).astype(bf16)
    sinq = np.ascontiguousarray(
        sin.reshape(NQ, SC, DH).transpose(0, 2, 1)).astype(bf16)

    P = np.zeros((DH, DH), np.float32)
    idx = np.arange(0, DH, 2)
    P[idx, idx + 1] = -1.0    # out[2i]   = -x[2i+1]
    P[idx + 1, idx] = 1.0     # out[2i+1] =  x[2i]
    PT = np.ascontiguousarray(P.T)

    # packed bf16 consts: PT | ones | zeros | tri(keep jj >= i)
    tri = (np.arange(128)[None, :] >= np.arange(128)[:, None])
    cb16 = np.concatenate(
        [PT, np.ones((128, 128), np.float32),
         np.zeros((128, 128), np.float32), tri.astype(np.float32)],
        axis=1).astype(bf16)

    # [NQ, 128, NT, SC]: xTq[qi, p, d, s] = x[b][qi*SC+s, d*128+p]
    xTqb = [np.ascontiguousarray(
        x[b].reshape(NQ, SC, NT, 128).transpose(0, 3, 2, 1)).astype(bf16)
        for b in range(B)]

    in_maps = []
    for core in range(N_CORES):
        b, g = divmod(core, G)
        c0 = g * CW
        # [128, NT, CW]: wqb[p, d, c] = wq[c0+c, d*128+p]
        wqb = np.ascontiguousarray(
            wq[c0:c0 + CW, :].reshape(CW, NT, 128).transpose(2, 1, 0)
        ).astype(bf16)
        wkb = np.ascontiguousarray(
            wk[c0:c0 + CW, :].reshape(CW, NT, 128).transpose(2, 1, 0)
        ).astype(bf16)
        wvb = np.ascontiguousarray(
            wv[c0:c0 + CW, :].reshape(CW, NT, 128).transpose(2, 1, 0)
        ).astype(bf16)
        # [128, HPG, D]: wob[p, h, j] = wo[j, c0+h*128+p]
        wob = np.ascontiguousarray(
            wo[:, c0:c0 + CW].reshape(D, HPG, 128).transpose(2, 1, 0)
        ).astype(bf16)
        # packed f32 consts: bvb (broadcast) | bq columns | bk columns
        cf32 = np.zeros((128, CW + 2 * HPG), np.float32)
        cf32[:, 0:CW] = bv[c0:c0 + CW][None, :]
        cf32[:, CW:CW + HPG] = bq[c0:c0 + CW].reshape(HPG, DH).T
        cf32[:, CW + HPG:] = bk[c0:c0 + CW].reshape(HPG, DH).T
        in_maps.append({
            "xTq": xTqb[b],
            "wqb": wqb,
            "wkb": wkb,
            "wvb": wvb,
            "wob": wob,
            "cf32d": cf32,
            "cb16d": cb16,
            "cosq": cosq,
            "sinq": sinq,
        })
    return in_maps


def _get_nc():
    if "nc" not in _NC_CACHE:
        _NC_CACHE["nc"] = build_attn_nc(iters=1)
    return _NC_CACHE["nc"]


def kernel(**inputs) -> np.ndarray:
    from concourse.bass_utils import run_bass_kernel_spmd

    nc = _get_nc()
    in_maps = host_prep(inputs)
    res = run_bass_kernel_spmd(nc, in_maps, core_ids=list(range(N_CORES)))
    bo = np.asarray(inputs["bo"], dtype=np.float32)
    outp = np.zeros((B, S, D), np.float32)
    for core in range(N_CORES):
        outp[core // G] += np.asarray(res.results[core]["out"],
                                      dtype=np.float32)
    outp += bo[None, None, :]
    return outp


# revision 40
# speedup vs baseline: 1.0228x; 1.0168x over previous
"""Trainium2 Bass kernel for 16-head causal self-attention with RoPE.

Problem: x:[2,2048,2048] -> MHA(wq,wk,wv,wo, causal mask, RoPE) -> [2,2048,2048].

Sharding (8 NeuronCores): core = b*4 + g, where b in {0,1} is the batch
(data parallel) and g in {0..3} is a head group of 4 heads (tensor parallel
over the 16 heads / 2048 channels: group g owns channels [g*512, (g+1)*512)).

v2 design (all intermediates SBUF-resident, bf16 matmuls):
  - x and all weights are pre-converted to bf16 on the host and loaded in a
    few large DMAs; q/k (RoPE'd, [dh, S] per head) and v ([S, cw]) live in
    SBUF for the whole kernel - no DRAM roundtrip between phases.
  - The S dimension is processed in 4 chunks of 512; for each chunk qi the
    QKV projections + RoPE (phase A) are immediately followed by the
    attention block for query chunk c=qi (phase B), which by causality only
    needs k/v chunks <= qi.  This interleaving spreads the ScalarE exp work
    and DVE work across the whole runtime so the Tensor engine stays busy.
  - softmax denominator: at-tiles are accumulated on DVE (bf16, 2x mode)
    into one [128,512] tile per (head, chunk); a single ones-matmul then
    reduces it across partitions (instead of one ones-matmul per sk-tile).
  - causal diagonal tiles are widened to >=256 free columns (f32r/bf16
    matmuls below 256 moving columns run at 1/4 rate on the PE); the
    triangular mask is applied in-place on the exp output (bf16 2x DVE).
  - RoPE rotate-half is a [128,128] P-matrix matmul (cross-partition swap
    can't be done on DVE); praw = psum+bias goes through ScalarE, the two
    sin/cos muls run on DVE and the final add runs on the Pool engine.
  - phase C (out = ao @ wo_loc.T) streams psum->SBUF copies through ScalarE
    and DMAs each [128,512] slice out as soon as it lands.
Host: out[b] = sum of the 4 group partials + bo.
"""

import math
import sys

sys.path.insert(0, "/opt/trn_rl_repo")

import numpy as np

N_CORES = 8
B, S, D = 2, 2048, 2048
H, DH = 16, 128
G = 4                 # head groups (tensor-parallel factor per batch)
HPG = H // G          # heads per group = 4
CW = HPG * DH         # channels per group = 512
NT = S // 128         # 16 d-tiles of the contraction dim
SC = 512              # free-dim chunk (one PSUM bank of fp32)
NQ = S // SC          # 4 s-chunks

_NC_CACHE: dict = {}


def build_attn_nc(iters: int = 1, phases: int = 3):
    """Build + compile the Bass module (same program for all 8 cores)."""
    import concourse.tile as tile
    from concourse import bacc, mybir

    f32 = mybir.dt.float32
    bf16 = mybir.dt.bfloat16
    AF = mybir.ActivationFunctionType
    SCALE = 1.0 / math.sqrt(DH)

    nc = bacc.Bacc("TRN2", target_bir_lowering=False, debug=False,
                   num_devices=N_CORES)

    # host-pre-laid-out inputs (see host_prep)
    xTq = nc.dram_tensor("xTq", [NQ, 128, NT, SC], bf16,
                         kind="ExternalInput").ap()
    wqb = nc.dram_tensor("wqb", [128, NT, CW], bf16, kind="ExternalInput").ap()
    wkb = nc.dram_tensor("wkb", [128, NT, CW], bf16, kind="ExternalInput").ap()
    wvb = nc.dram_tensor("wvb", [128, NT, CW], bf16, kind="ExternalInput").ap()
    wob = nc.dram_tensor("wob", [128, HPG, D], bf16, kind="ExternalInput").ap()
    # packed constants (single DMA each): cf32 = bvb | bq | bk,
    # cb16 = PT | ones | mask(zeros|tri)
    cf32d = nc.dram_tensor("cf32d", [128, CW + 2 * HPG], f32,
                           kind="ExternalInput").ap()
    cb16d = nc.dram_tensor("cb16d", [128, 512], bf16,
                           kind="ExternalInput").ap()
    cosq = nc.dram_tensor("cosq", [NQ, DH, SC], bf16,
                          kind="ExternalInput").ap()
    sinq = nc.dram_tensor("sinq", [NQ, DH, SC], bf16,
                          kind="ExternalInput").ap()

    out = nc.dram_tensor("out", [S, D], bf16, kind="ExternalOutput").ap()

    with tile.TileContext(nc) as tc:
        for it in range(iters):
            with tc.tile_pool(name="const", bufs=1) as cpool, \
                 tc.tile_pool(name="wts", bufs=1) as wpool, \
                 tc.tile_pool(name="perst", bufs=1) as ppool:
                # ---- persistent SBUF tensors --------------------------
                qT = [ppool.tile([DH, S], bf16, name=f"qT{h}_{it}",
                                 tag=f"qT{h}") for h in range(HPG)]
                kT = [ppool.tile([DH, S], bf16, name=f"kT{h}_{it}",
                                 tag=f"kT{h}") for h in range(HPG)]
                v_t = [ppool.tile([128, CW], bf16, name=f"v{t}_{it}",
                                  tag=f"v{t}") for t in range(NT)]
                aoT = ppool.tile([128, HPG * S], bf16, name=f"aoT_{it}",
                                 tag="aoT")

                # ---- startup DMAs (order = queue service order) -------
                # scalar queue: wq parts first so the first matmul can start
                # ~2us in, then the small consts, then wk/wv/wo.
                # sync queue: x chunk 0 + cos/sin.
                w_sb = {}
                for nm, dram in (("q", wqb),):
                    t = wpool.tile([128, NT, CW], bf16, name=f"w{nm}_{it}",
                                   tag=f"w{nm}")
                    for j0, j1 in ((0, 1), (1, 2), (2, 4), (4, 8), (8, 16)):
                        nc.scalar.dma_start(t[:, j0:j1, :],
                                            dram[:, j0:j1, :])
                    w_sb[nm] = t
                cb16 = cpool.tile([128, 512], bf16, name=f"cb16{it}",
                                  tag="cb16")
                nc.scalar.dma_start(cb16[:], cb16d[:])
                cf32 = cpool.tile([128, CW + 2 * HPG], f32,
                                  name=f"cf32{it}", tag="cf32")
                nc.scalar.dma_start(cf32[:], cf32d[:])
                pt_sb = cb16[:, 0:DH]
                ones_sb = cb16[:, 128:256]
                msk_full = cb16[:, 256:512]   # [zeros(128) | tri(128)]
                msk_tri = cb16[:, 384:512]
                bvb_sb = cf32[:, 0:CW]
                bq_sb = [cf32[:, CW + ct:CW + ct + 1] for ct in range(HPG)]
                bk_sb = [cf32[:, CW + HPG + ct:CW + HPG + ct + 1]
                         for ct in range(HPG)]
                # wk/wv/wo arrive during compute: one large DMA each
                # minimizes shared-pipe overhead
                for nm, dram in (("k", wkb), ("v", wvb)):
                    t = wpool.tile([128, NT, CW], bf16, name=f"w{nm}_{it}",
                                   tag=f"w{nm}")
                    nc.scalar.dma_start(t[:], dram[:])
                    w_sb[nm] = t
                wo_sb = wpool.tile([128, HPG, D], bf16, name=f"wo{it}",
                                   tag="wo")
                nc.scalar.dma_start(wo_sb[:], wob[:])

                with tc.tile_pool(name="xqp", bufs=2) as xqpool, \
                     tc.tile_pool(name="csp", bufs=2) as cspool, \
                     tc.tile_pool(name="prawp", bufs=2) as prawp, \
                     tc.tile_pool(name="workA", bufs=2) as wkp, \
                     tc.tile_pool(name="atp", bufs=6) as atpool, \
                     tc.tile_pool(name="accp", bufs=2) as accpool, \
                     tc.tile_pool(name="recp", bufs=2) as recpool, \
                     tc.tile_pool(name="outp", bufs=6) as outpool, \
                     tc.tile_pool(name="psA", bufs=3, space="PSUM") as psA, \
                     tc.tile_pool(name="psT", bufs=3, space="PSUM") as psT, \
                     tc.tile_pool(name="psO", bufs=2, space="PSUM") as psO:

                    def load_chunk(qi):
                        xq = xqpool.tile([128, NT, SC], bf16,
                                         name=f"xq{qi}_{it}", tag="xq")
                        cos_c = cspool.tile([DH, SC], bf16,
                                            name=f"cos{qi}_{it}", tag="cos")
                        sin_c = cspool.tile([DH, SC], bf16,
                                            name=f"sin{qi}_{it}", tag="sin")
                        if qi == 0:
                            # fine ramped parts: first matmul starts early
                            for j0, j1 in ((0, 1), (1, 2), (2, 4), (4, 8),
                                           (8, 16)):
                                nc.sync.dma_start(xq[:, j0:j1, :],
                                                  xTq[qi][:, j0:j1, :])
                        else:
                            nc.sync.dma_start(xq[:], xTq[qi])
                        nc.sync.dma_start(cos_c[:], cosq[qi])
                        nc.sync.dma_start(sin_c[:], sinq[qi])
                        return xq, cos_c, sin_c

                    def c_block_st(st):
                        # one out-projection s-tile; interleaved between
                        # attention head-blocks so its matmuls fill PE gaps
                        # left by the exp-bound B-blocks.
                        for dcp in range(0, 4, 2):
                            psa = psA.tile([128, SC], f32,
                                           name=f"op{st}{dcp}_{it}",
                                           tag="ps")
                            psb = psO.tile([128, SC], f32,
                                           name=f"op{st}{dcp+1}_{it}",
                                           tag="oT")
                            for h in range(HPG):
                                lhs = aoT[:, h * S + st * 128:
                                          h * S + (st + 1) * 128]
                                nc.tensor.matmul(
                                    psa[:], lhs,
                                    wo_sb[:, h, dcp * SC:(dcp + 1) * SC],
                                    start=(h == 0), stop=(h == HPG - 1))
                                nc.tensor.matmul(
                                    psb[:], lhs,
                                    wo_sb[:, h,
                                          (dcp + 1) * SC:(dcp + 2) * SC],
                                    start=(h == 0), stop=(h == HPG - 1))
                            for dc, op in ((dcp, psa), (dcp + 1, psb)):
                                ot = outpool.tile([128, SC], bf16,
                                                  name=f"ot{st}{dc}_{it}",
                                                  tag="ot")
                                if dc % 2 == 0:
                                    nc.scalar.activation(ot[:], op[:],
                                                         AF.Copy)
                                else:
                                    nc.vector.tensor_copy(ot[:], op[:])
                                nc.sync.dma_start(
                                    out[st * 128:(st + 1) * 128,
                                        dc * SC:(dc + 1) * SC], ot[:])

                    C_FILL = {(1, 0): [0], (1, 1): [1],
                              (2, 0): [2], (2, 1): [3], (2, 2): [4],
                              (3, 0): [5, 6], (3, 1): [7, 8],
                              (3, 2): [9, 10], (3, 3): [11]}
                    xq, cos_c, sin_c = load_chunk(0)
                    for qi in range(NQ):
                        # ---- phase A: q/k projections + RoPE ----------
                        for nm, bias_sb, qkT in (("q", bq_sb, qT),
                                                 ("k", bk_sb, kT)):
                            for cp in range(0, HPG, 2):
                                psa = psA.tile([128, SC], f32,
                                               name=f"ps{nm}{cp}_{qi}_{it}",
                                               tag="ps")
                                psb = psA.tile([128, SC], f32,
                                               name=f"ps{nm}{cp+1}_{qi}_{it}",
                                               tag="ps")
                                for d in range(NT):
                                    nc.tensor.matmul(
                                        psa[:],
                                        w_sb[nm][:, d, cp * DH:(cp + 1) * DH],
                                        xq[:, d, :],
                                        start=(d == 0), stop=(d == NT - 1))
                                    nc.tensor.matmul(
                                        psb[:],
                                        w_sb[nm][:, d,
                                                 (cp + 1) * DH:(cp + 2) * DH],
                                        xq[:, d, :],
                                        start=(d == 0), stop=(d == NT - 1))
                                for ct, ps in ((cp, psa), (cp + 1, psb)):
                                    praw = prawp.tile(
                                        [128, SC], bf16,
                                        name=f"praw{nm}{ct}_{qi}_{it}",
                                        tag=f"praw{ct}")
                                    nc.vector.tensor_scalar_add(
                                        praw[:], ps[:], bias_sb[ct])
                                    psr = psT.tile([128, SC], f32,
                                                   name=f"psr{nm}{ct}_{qi}_{it}",
                                                   tag="tr")
                                    nc.tensor.matmul(psr[:], pt_sb,
                                                     praw[:],
                                                     start=True, stop=True)
                                    m1 = wkp.tile([128, SC], bf16,
                                                  name=f"m1{nm}{ct}_{qi}_{it}",
                                                  tag="m1")
                                    nc.vector.tensor_mul(m1[:], praw[:],
                                                         cos_c[:])
                                    m2 = wkp.tile([128, SC], bf16,
                                                  name=f"m2{nm}{ct}_{qi}_{it}",
                                                  tag="m2")
                                    nc.vector.tensor_mul(m2[:], psr[:],
                                                         sin_c[:])
                                    nc.gpsimd.tensor_add(
                                        qkT[ct][:, qi * SC:(qi + 1) * SC],
                                        m1[:], m2[:])

                        # ---- phase A: v projection --------------------
                        for sp in range(0, 4, 2):
                            psa = psA.tile([128, SC], f32,
                                           name=f"psv{sp}_{qi}_{it}",
                                           tag="ps")
                            psb = psA.tile([128, SC], f32,
                                           name=f"psv{sp+1}_{qi}_{it}",
                                           tag="ps")
                            for d in range(NT):
                                nc.tensor.matmul(
                                    psa[:],
                                    xq[:, d, sp * 128:(sp + 1) * 128],
                                    w_sb["v"][:, d, :],
                                    start=(d == 0), stop=(d == NT - 1))
                                nc.tensor.matmul(
                                    psb[:],
                                    xq[:, d, (sp + 1) * 128:(sp + 2) * 128],
                                    w_sb["v"][:, d, :],
                                    start=(d == 0), stop=(d == NT - 1))
                            for st, ps in ((sp, psa), (sp + 1, psb)):
                                nc.vector.tensor_add(v_t[qi * 4 + st][:],
                                                     ps[:], bvb_sb)

                        # prefetch next x chunk before this block's out DMAs
                        if qi + 1 < NQ:
                            nxt = load_chunk(qi + 1)

                        # ---- phase B: attention for query chunk c=qi --
                        c = qi
                        ntile = 4 * c + 4
                        for h in range(HPG):
                            oT = psO.tile([DH, SC], f32,
                                          name=f"oT{h}{c}_{it}", tag="oT")
                            acc = accpool.tile([128, SC], bf16,
                                               name=f"acc{h}{c}_{it}",
                                               tag="acc")
                            for t_ in range(ntile):
                                rr = t_ - 4 * c
                                n0 = max(rr, 0) * 128
                                sps = psT.tile([128, SC], f32,
                                               name=f"sps{h}{c}{t_}_{it}",
                                               tag="tr")
                                nc.tensor.matmul(
                                    sps[:, n0:],
                                    kT[h][:, t_ * 128:(t_ + 1) * 128],
                                    qT[h][:, c * SC + n0:(c + 1) * SC],
                                    start=True, stop=True)
                                # tile 0's exp goes straight into the
                                # denominator accumulator (saves a copy);
                                # later tiles' adds WAR-wait on its PV read
                                if t_ == 0:
                                    at = acc
                                else:
                                    at = atpool.tile([128, SC], bf16,
                                                     name=f"at{h}{c}{t_}_{it}",
                                                     tag="at")
                                nc.scalar.activation(at[:, n0:], sps[:, n0:],
                                                     AF.Exp, bias=0.0,
                                                     scale=SCALE)
                                if rr >= 0:
                                    blk = slice(rr * 128, (rr + 1) * 128)
                                    nc.vector.tensor_mul(
                                        at[:, blk], at[:, blk], msk_tri)
                                nc.tensor.matmul(
                                    oT[:, n0:],
                                    v_t[t_][:, h * DH:(h + 1) * DH],
                                    at[:, n0:],
                                    start=(t_ == 0), stop=(t_ == ntile - 1),
                                    skip_group_check=True)
                                if t_ > 0:
                                    nc.vector.tensor_add(acc[:, n0:],
                                                         acc[:, n0:],
                                                         at[:, n0:])
                            dnp = psT.tile([128, SC], f32,
                                           name=f"dn{h}{c}_{it}", tag="tr")
                            nc.tensor.matmul(dnp[:], ones_sb, acc[:],
                                             start=True, stop=True)
                            rec = recpool.tile([128, SC], f32,
                                               name=f"rec{h}{c}_{it}",
                                               tag="rec")
                            nc.vector.reciprocal(rec[:], dnp[:])
                            nc.vector.tensor_mul(
                                aoT[:, h * S + c * SC:h * S + (c + 1) * SC],
                                oT[:], rec[:])
                            # earlier chunks' out-projection tiles as PE
                            # gap fillers, weighted toward the exp-heavy
                            # late chunks
                            for st in C_FILL.get((qi, h), ()):
                                c_block_st(st)
                        if qi + 1 < NQ:
                            xq, cos_c, sin_c = nxt

                    # ---- tail: out-projection for the last chunk ------
                    for st in range(4 * (NQ - 1), 4 * NQ):
                        c_block_st(st)
    nc.compile()
    return nc


def host_prep(inputs: dict) -> list:
    """Build per-core input maps (host-side sharding + bf16 relayout)."""
    import ml_dtypes
    bf16 = ml_dtypes.bfloat16

    x = np.asarray(inputs["x"], dtype=np.float32)
    wq = np.asarray(inputs["wq"], dtype=np.float32)
    wk = np.asarray(inputs["wk"], dtype=np.float32)
    wv = np.asarray(inputs["wv"], dtype=np.float32)
    wo = np.asarray(inputs["wo"], dtype=np.float32)
    bq = np.asarray(inputs["bq"], dtype=np.float32)
    bk = np.asarray(inputs["bk"], dtype=np.float32)
    bv = np.asarray(inputs["bv"], dtype=np.float32)

    inv = 1.0 / (10000.0 ** (np.arange(0, DH, 2, dtype=np.float64) / DH))
    ang = np.arange(S, dtype=np.float64)[:, None] * inv[None, :]
    sin = np.repeat(np.sin(ang), 2, axis=1).astype(np.float32)  # [S, DH]
    cos = np.repeat(np.cos(ang), 2, axis=1).astype(np.float32)
    # [NQ, DH, SC]: cosq[qi, p, s] = cos[qi*SC+s, p]
    cosq = np.ascontiguousarray(
        cos.reshape(NQ, SC, DH).transpose(0, 2, 1)).astype(bf16)
    sinq = np.ascontiguousarray(
        sin.reshape(NQ, SC, DH).transpose(0, 2, 1)).astype(bf16)

    P = np.zeros((DH, DH), np.float32)
    idx = np.arange(0, DH, 2)
    P[idx, idx + 1] = -1.0    # out[2i]   = -x[2i+1]
    P[idx + 1, idx] = 1.0     # out[2i+1] =  x[2i]
    PT = np.ascontiguousarray(P.T)

    # packed bf16 consts: PT | ones | zeros | tri(keep jj >= i)
    tri = (np.arange(128)[None, :] >= np.arange(128)[:, None])
    cb16 = np.concatenate(
        [PT, np.ones((128, 128), np.float32),
         np.zeros((128, 128), np.float32), tri.astype(np.float32)],
        axis=1).astype(bf16)

    # [NQ, 128, NT, SC]: xTq[qi, p, d, s] = x[b][qi*SC+s, d*128+p]
    xTqb = [np.ascontiguousarray(
        x[b].reshape(NQ, SC, NT, 128).transpose(0, 3, 2, 1)).astype(bf16)
        for b in range(B)]

    in_maps = []
    for core in range(N_CORES):
        b, g = divmod(core, G)
        c0 = g * CW
        # [128, NT, CW]: wqb[p, d, c] = wq[c0+c, d*128+p]
        wqb = np.ascontiguousarray(
            wq[c0:c0 + CW, :].reshape(CW, NT, 128).transpose(2, 1, 0)
        ).astype(bf16)
        wkb = np.ascontiguousarray(
            wk[c0:c0 + CW, :].reshape(CW, NT, 128).transpose(2, 1, 0)
        ).astype(bf16)
        wvb = np.ascontiguousarray(
            wv[c0:c0 + CW, :].reshape(CW, NT, 128).transpose(2, 1, 0)
        ).astype(bf16)
        # [128, HPG, D]: wob[p, h, j] = wo[j, c0+h*128+p]
        wob = np.ascontiguousarray(
            wo[:, c0:c0 + CW].reshape(D, HPG, 128).transpose(2, 1, 0)
        ).astype(bf16)
        # packed f32 consts: bvb (broadcast) | bq columns | bk columns
        cf32 = np.zeros((128, CW + 2 * HPG), np.float32)
        cf32[:, 0:CW] = bv[c0:c0 + CW][None, :]
        cf32[:, CW:CW + HPG] = bq[c0:c0 + CW].reshape(HPG, DH).T
        cf32[:, CW + HPG:] = bk[c0:c0 + CW].reshape(HPG, DH).T
        in_maps.append({
            "xTq": xTqb[b],
            "wqb": wqb,
            "wkb": wkb,
            "wvb": wvb,
            "wob": wob,
            "cf32d": cf32,
            "cb16d": cb16,
            "cosq": cosq,
            "sinq": sinq,
        })
    return in_maps


def _get_nc():
    if "nc" not in _NC_CACHE:
        _NC_CACHE["nc"] = build_attn_nc(iters=1)
    return _NC_CACHE["nc"]


def kernel(**inputs) -> np.ndarray:
    from concourse.bass_utils import run_bass_kernel_spmd

    nc = _get_nc()
    in_maps = host_prep(inputs)
    res = run_bass_kernel_spmd(nc, in_maps, core_ids=list(range(N_CORES)))
    bo = np.asarray(inputs["bo"], dtype=np.float32)
    outp = np.zeros((B, S, D), np.float32)
    for core in range(N_CORES):
        outp[core // G] += np.asarray(res.results[core]["out"],
                                      dtype=np.float32)
    outp += bo[None, None, :]
    return outp


# revision 45
# speedup vs baseline: 1.0257x; 1.0028x over previous
"""Trainium2 Bass kernel for 16-head causal self-attention with RoPE.

Problem: x:[2,2048,2048] -> MHA(wq,wk,wv,wo, causal mask, RoPE) -> [2,2048,2048].

Sharding (8 NeuronCores): core = b*4 + g, where b in {0,1} is the batch
(data parallel) and g in {0..3} is a head group of 4 heads (tensor parallel
over the 16 heads / 2048 channels: group g owns channels [g*512, (g+1)*512)).

v2 design (all intermediates SBUF-resident, bf16 matmuls):
  - x and all weights are pre-converted to bf16 on the host and loaded in a
    few large DMAs; q/k (RoPE'd, [dh, S] per head) and v ([S, cw]) live in
    SBUF for the whole kernel - no DRAM roundtrip between phases.
  - The S dimension is processed in 4 chunks of 512; for each chunk qi the
    QKV projections + RoPE (phase A) are immediately followed by the
    attention block for query chunk c=qi (phase B), which by causality only
    needs k/v chunks <= qi.  This interleaving spreads the ScalarE exp work
    and DVE work across the whole runtime so the Tensor engine stays busy.
  - softmax denominator: at-tiles are accumulated on DVE (bf16, 2x mode)
    into one [128,512] tile per (head, chunk); a single ones-matmul then
    reduces it across partitions (instead of one ones-matmul per sk-tile).
  - causal diagonal tiles are widened to >=256 free columns (f32r/bf16
    matmuls below 256 moving columns run at 1/4 rate on the PE); the
    triangular mask is applied in-place on the exp output (bf16 2x DVE).
  - RoPE rotate-half is a [128,128] P-matrix matmul (cross-partition swap
    can't be done on DVE); praw = psum+bias goes through ScalarE, the two
    sin/cos muls run on DVE and the final add runs on the Pool engine.
  - phase C (out = ao @ wo_loc.T) streams psum->SBUF copies through ScalarE
    and DMAs each [128,512] slice out as soon as it lands.
Host: out[b] = sum of the 4 group partials + bo.
"""

import math
import sys

sys.path.insert(0, "/opt/trn_rl_repo")

import numpy as np

N_CORES = 8
B, S, D = 2, 2048, 2048
H, DH = 16, 128
G = 4                 # head groups (tensor-parallel factor per batch)
HPG = H // G          # heads per group = 4
CW = HPG * DH         # channels per group = 512
NT = S // 128         # 16 d-tiles of the contraction dim
SC = 512              # free-dim chunk (one PSUM bank of fp32)
NQ = S // SC          # 4 s-chunks

_NC_CACHE: dict = {}


def build_attn_nc(iters: int = 1, phases: int = 3):
    """Build + compile the Bass module (same program for all 8 cores)."""
    import concourse.tile as tile
    from concourse import bacc, mybir

    f32 = mybir.dt.float32
    bf16 = mybir.dt.bfloat16
    AF = mybir.ActivationFunctionType
    SCALE = 1.0 / math.sqrt(DH)

    nc = bacc.Bacc("TRN2", target_bir_lowering=False, debug=False,
                   num_devices=N_CORES)

    # host-pre-laid-out inputs (see host_prep)
    xTq = nc.dram_tensor("xTq", [NQ, 128, NT, SC], bf16,
                         kind="ExternalInput").ap()
    wqb = nc.dram_tensor("wqb", [128, NT, CW], bf16, kind="ExternalInput").ap()
    wkb = nc.dram_tensor("wkb", [128, NT, CW], bf16, kind="ExternalInput").ap()
    wvb = nc.dram_tensor("wvb", [128, NT, CW], bf16, kind="ExternalInput").ap()
    wob = nc.dram_tensor("wob", [128, HPG, D], bf16, kind="ExternalInput").ap()
    # packed constants (single DMA each): cf32 = bvb | bq | bk,
    # cb16 = PT | ones | mask(zeros|tri)
    cf32d = nc.dram_tensor("cf32d", [128, CW + 2 * HPG], f32,
                           kind="ExternalInput").ap()
    cb16d = nc.dram_tensor("cb16d", [128, 512], bf16,
                           kind="ExternalInput").ap()
    cosq = nc.dram_tensor("cosq", [NQ, DH, SC], bf16,
                          kind="ExternalInput").ap()
    sinq = nc.dram_tensor("sinq", [NQ, DH, SC], bf16,
                          kind="ExternalInput").ap()

    out = nc.dram_tensor("out", [S, D], bf16, kind="ExternalOutput").ap()

    with tile.TileContext(nc) as tc:
        for it in range(iters):
            with tc.tile_pool(name="const", bufs=1) as cpool, \
                 tc.tile_pool(name="wts", bufs=1) as wpool, \
                 tc.tile_pool(name="perst", bufs=1) as ppool:
                # ---- persistent SBUF tensors --------------------------
                qT = [ppool.tile([DH, S], bf16, name=f"qT{h}_{it}",
                                 tag=f"qT{h}") for h in range(HPG)]
                kT = [ppool.tile([DH, S], bf16, name=f"kT{h}_{it}",
                                 tag=f"kT{h}") for h in range(HPG)]
                v_t = [ppool.tile([128, CW], bf16, name=f"v{t}_{it}",
                                  tag=f"v{t}") for t in range(NT)]
                aoT = ppool.tile([128, HPG * S], bf16, name=f"aoT_{it}",
                                 tag="aoT")

                # ---- startup DMAs (order = queue service order) -------
                # scalar queue: wq parts first so the first matmul can start
                # ~2us in, then the small consts, then wk/wv/wo.
                # sync queue: x chunk 0 + cos/sin.
                w_sb = {}
                for nm, dram in (("q", wqb),):
                    t = wpool.tile([128, NT, CW], bf16, name=f"w{nm}_{it}",
                                   tag=f"w{nm}")
                    for j0, j1 in ((0, 1), (1, 2), (2, 4), (4, 8), (8, 16)):
                        nc.scalar.dma_start(t[:, j0:j1, :],
                                            dram[:, j0:j1, :])
                    w_sb[nm] = t
                cb16 = cpool.tile([128, 512], bf16, name=f"cb16{it}",
                                  tag="cb16")
                nc.scalar.dma_start(cb16[:], cb16d[:])
                cf32 = cpool.tile([128, CW + 2 * HPG], f32,
                                  name=f"cf32{it}", tag="cf32")
                nc.scalar.dma_start(cf32[:], cf32d[:])
                pt_sb = cb16[:, 0:DH]
                ones_sb = cb16[:, 128:256]
                msk_full = cb16[:, 256:512]   # [zeros(128) | tri(128)]
                msk_tri = cb16[:, 384:512]
                bvb_sb = cf32[:, 0:CW]
                bq_sb = [cf32[:, CW + ct:CW + ct + 1] for ct in range(HPG)]
                bk_sb = [cf32[:, CW + HPG + ct:CW + HPG + ct + 1]
                         for ct in range(HPG)]
                # wk/wv/wo arrive during compute: one large DMA each
                # minimizes shared-pipe overhead
                for nm, dram in (("k", wkb), ("v", wvb)):
                    t = wpool.tile([128, NT, CW], bf16, name=f"w{nm}_{it}",
                                   tag=f"w{nm}")
                    nc.scalar.dma_start(t[:], dram[:])
                    w_sb[nm] = t
                wo_sb = wpool.tile([128, HPG, D], bf16, name=f"wo{it}",
                                   tag="wo")
                nc.scalar.dma_start(wo_sb[:], wob[:])

                with tc.tile_pool(name="xqp", bufs=2) as xqpool, \
                     tc.tile_pool(name="csp", bufs=2) as cspool, \
                     tc.tile_pool(name="prawp", bufs=2) as prawp, \
                     tc.tile_pool(name="workA", bufs=2) as wkp, \
                     tc.tile_pool(name="atp", bufs=8) as atpool, \
                     tc.tile_pool(name="accp", bufs=3) as accpool, \
                     tc.tile_pool(name="recp", bufs=3) as recpool, \
                     tc.tile_pool(name="outp", bufs=6) as outpool, \
                     tc.tile_pool(name="psA", bufs=3, space="PSUM") as psA, \
                     tc.tile_pool(name="psT", bufs=3, space="PSUM") as psT, \
                     tc.tile_pool(name="psO", bufs=2, space="PSUM") as psO:

                    def load_chunk(qi):
                        xq = xqpool.tile([128, NT, SC], bf16,
                                         name=f"xq{qi}_{it}", tag="xq")
                        cos_c = cspool.tile([DH, SC], bf16,
                                            name=f"cos{qi}_{it}", tag="cos")
                        sin_c = cspool.tile([DH, SC], bf16,
                                            name=f"sin{qi}_{it}", tag="sin")
                        if qi == 0:
                            # fine ramped parts: first matmul starts early
                            for j0, j1 in ((0, 1), (1, 2), (2, 4), (4, 8),
                                           (8, 16)):
                                nc.sync.dma_start(xq[:, j0:j1, :],
                                                  xTq[qi][:, j0:j1, :])
                        else:
                            nc.sync.dma_start(xq[:], xTq[qi])
                        nc.sync.dma_start(cos_c[:], cosq[qi])
                        nc.sync.dma_start(sin_c[:], sinq[qi])
                        return xq, cos_c, sin_c

                    def c_block_st(st):
                        # one out-projection s-tile; interleaved between
                        # attention head-blocks so its matmuls fill PE gaps
                        # left by the exp-bound B-blocks.
                        for dcp in range(0, 4, 2):
                            psa = psA.tile([128, SC], f32,
                                           name=f"op{st}{dcp}_{it}",
                                           tag="ps")
                            psb = psO.tile([128, SC], f32,
                                           name=f"op{st}{dcp+1}_{it}",
                                           tag="oT")
                            for h in range(HPG):
                                lhs = aoT[:, h * S + st * 128:
                                          h * S + (st + 1) * 128]
                                nc.tensor.matmul(
                                    psa[:], lhs,
                                    wo_sb[:, h, dcp * SC:(dcp + 1) * SC],
                                    start=(h == 0), stop=(h == HPG - 1))
                                nc.tensor.matmul(
                                    psb[:], lhs,
                                    wo_sb[:, h,
                                          (dcp + 1) * SC:(dcp + 2) * SC],
                                    start=(h == 0), stop=(h == HPG - 1))
                            for dc, op in ((dcp, psa), (dcp + 1, psb)):
                                ot = outpool.tile([128, SC], bf16,
                                                  name=f"ot{st}{dc}_{it}",
                                                  tag="ot")
                                if dc % 2 == 0:
                                    nc.scalar.activation(ot[:], op[:],
                                                         AF.Copy)
                                else:
                                    nc.vector.tensor_copy(ot[:], op[:])
                                nc.sync.dma_start(
                                    out[st * 128:(st + 1) * 128,
                                        dc * SC:(dc + 1) * SC], ot[:])

                    C_FILL = {(1, 0): [0], (1, 1): [1],
                              (2, 0): [2], (2, 1): [3], (2, 2): [4],
                              (3, 0): [5, 6], (3, 1): [7, 8],
                              (3, 2): [9, 10], (3, 3): [11]}
                    xq, cos_c, sin_c = load_chunk(0)
                    for qi in range(NQ):
                        # ---- phase A: q/k projections + RoPE ----------
                        def proj_qk(nm, bias_sb, qkT):
                            for cp in range(0, HPG, 2):
                                psa = psA.tile([128, SC], f32,
                                               name=f"ps{nm}{cp}_{qi}_{it}",
                                               tag="ps")
                                psb = psA.tile([128, SC], f32,
                                               name=f"ps{nm}{cp+1}_{qi}_{it}",
                                               tag="ps")
                                for d in range(NT):
                                    nc.tensor.matmul(
                                        psa[:],
                                        w_sb[nm][:, d, cp * DH:(cp + 1) * DH],
                                        xq[:, d, :],
                                        start=(d == 0), stop=(d == NT - 1))
                                    nc.tensor.matmul(
                                        psb[:],
                                        w_sb[nm][:, d,
                                                 (cp + 1) * DH:(cp + 2) * DH],
                                        xq[:, d, :],
                                        start=(d == 0), stop=(d == NT - 1))
                                for ct, ps in ((cp, psa), (cp + 1, psb)):
                                    praw = prawp.tile(
                                        [128, SC], bf16,
                                        name=f"praw{nm}{ct}_{qi}_{it}",
                                        tag=f"praw{ct}")
                                    nc.vector.tensor_scalar_add(
                                        praw[:], ps[:], bias_sb[ct])
                                    psr = psT.tile([128, SC], f32,
                                                   name=f"psr{nm}{ct}_{qi}_{it}",
                                                   tag="tr")
                                    nc.tensor.matmul(psr[:], pt_sb,
                                                     praw[:],
                                                     start=True, stop=True)
                                    m1 = wkp.tile([128, SC], bf16,
                                                  name=f"m1{nm}{ct}_{qi}_{it}",
                                                  tag="m1")
                                    nc.vector.tensor_mul(m1[:], praw[:],
                                                         cos_c[:])
                                    m2 = wkp.tile([128, SC], bf16,
                                                  name=f"m2{nm}{ct}_{qi}_{it}",
                                                  tag="m2")
                                    nc.vector.tensor_mul(m2[:], psr[:],
                                                         sin_c[:])
                                    nc.gpsimd.tensor_add(
                                        qkT[ct][:, qi * SC:(qi + 1) * SC],
                                        m1[:], m2[:])

                        def proj_v():
                            for sp in range(0, 4, 2):
                                psa = psA.tile([128, SC], f32,
                                               name=f"psv{sp}_{qi}_{it}",
                                               tag="ps")
                                psb = psA.tile([128, SC], f32,
                                               name=f"psv{sp+1}_{qi}_{it}",
                                               tag="ps")
                                for d in range(NT):
                                    nc.tensor.matmul(
                                        psa[:],
                                        xq[:, d, sp * 128:(sp + 1) * 128],
                                        w_sb["v"][:, d, :],
                                        start=(d == 0), stop=(d == NT - 1))
                                    nc.tensor.matmul(
                                        psb[:],
                                        xq[:, d,
                                           (sp + 1) * 128:(sp + 2) * 128],
                                        w_sb["v"][:, d, :],
                                        start=(d == 0), stop=(d == NT - 1))
                                for st, ps in ((sp, psa), (sp + 1, psb)):
                                    nc.vector.tensor_add(v_t[qi * 4 + st][:],
                                                         ps[:], bvb_sb)

                        # ---- phase B helpers: attention for chunk c=qi;
                        # tiles below 4c need only k/v chunks < c, so they
                        # can overlap the K/V projections of this chunk
                        c = qi
                        ntile = 4 * c + 4

                        def b_open(h):
                            oT = psO.tile([DH, SC], f32,
                                          name=f"oT{h}{c}_{it}", tag="oT")
                            acc = accpool.tile([128, SC], bf16,
                                               name=f"acc{h}{c}_{it}",
                                               tag="acc")
                            return oT, acc

                        def b_tiles(h, oT, acc, t_lo, t_hi):
                            for t_ in range(t_lo, t_hi):
                                rr = t_ - 4 * c
                                n0 = max(rr, 0) * 128
                                sps = psT.tile([128, SC], f32,
                                               name=f"sps{h}{c}{t_}_{it}",
                                               tag="tr")
                                nc.tensor.matmul(
                                    sps[:, n0:],
                                    kT[h][:, t_ * 128:(t_ + 1) * 128],
                                    qT[h][:, c * SC + n0:(c + 1) * SC],
                                    start=True, stop=True)
                                # tile 0's exp goes straight into the
                                # denominator accumulator (saves a copy);
                                # later tiles' adds WAR-wait on its PV read
                                if t_ == 0:
                                    at = acc
                                else:
                                    at = atpool.tile([128, SC], bf16,
                                                     name=f"at{h}{c}{t_}_{it}",
                                                     tag="at")
                                nc.scalar.activation(at[:, n0:], sps[:, n0:],
                                                     AF.Exp, bias=0.0,
                                                     scale=SCALE)
                                if rr >= 0:
                                    blk = slice(rr * 128, (rr + 1) * 128)
                                    nc.vector.tensor_mul(
                                        at[:, blk], at[:, blk], msk_tri)
                                nc.tensor.matmul(
                                    oT[:, n0:],
                                    v_t[t_][:, h * DH:(h + 1) * DH],
                                    at[:, n0:],
                                    start=(t_ == 0), stop=(t_ == ntile - 1),
                                    skip_group_check=True)
                                if t_ > 0:
                                    nc.vector.tensor_add(acc[:, n0:],
                                                         acc[:, n0:],
                                                         at[:, n0:])
                        def b_close(h, oT, acc):
                            dnp = psT.tile([128, SC], f32,
                                           name=f"dn{h}{c}_{it}", tag="tr")
                            nc.tensor.matmul(dnp[:], ones_sb, acc[:],
                                             start=True, stop=True)
                            rec = recpool.tile([128, SC], f32,
                                               name=f"rec{h}{c}_{it}",
                                               tag="rec")
                            nc.vector.reciprocal(rec[:], dnp[:])
                            nc.vector.tensor_mul(
                                aoT[:, h * S + c * SC:h * S + (c + 1) * SC],
                                oT[:], rec[:])
                            # earlier chunks' out-projection tiles as PE
                            # gap fillers, weighted toward the exp-heavy
                            # late chunks
                            for st in C_FILL.get((qi, h), ()):
                                c_block_st(st)

                        # ---- emission order: overlap early attention
                        # tiles (k/v < c) with this chunk's K/V projections
                        proj_qk("q", bq_sb, qT)
                        if c >= 1:
                            oT0, acc0 = b_open(0)
                            b_tiles(0, oT0, acc0, 0, 4 * c)
                            proj_qk("k", bk_sb, kT)
                            oT1, acc1 = b_open(1)
                            b_tiles(1, oT1, acc1, 0, 4 * c)
                            proj_v()
                            if qi + 1 < NQ:
                                nxt = load_chunk(qi + 1)
                            b_tiles(0, oT0, acc0, 4 * c, ntile)
                            b_close(0, oT0, acc0)
                            b_tiles(1, oT1, acc1, 4 * c, ntile)
                            b_close(1, oT1, acc1)
                            for h in (2, 3):
                                oTh, acch = b_open(h)
                                b_tiles(h, oTh, acch, 0, ntile)
                                b_close(h, oTh, acch)
                        else:
                            proj_qk("k", bk_sb, kT)
                            proj_v()
                            if qi + 1 < NQ:
                                nxt = load_chunk(qi + 1)
                            for h in range(HPG):
                                oTh, acch = b_open(h)
                                b_tiles(h, oTh, acch, 0, ntile)
                                b_close(h, oTh, acch)
                        if qi + 1 < NQ:
                            xq, cos_c, sin_c = nxt

                    # ---- tail: out-projection for the last chunk ------
                    for st in range(4 * (NQ - 1), 4 * NQ):
                        c_block_st(st)
    nc.compile()
    return nc


def host_prep(inputs: dict) -> list:
    """Build per-core input maps (host-side sharding + bf16 relayout)."""
    import ml_dtypes
    bf16 = ml_dtypes.bfloat16

    x = np.asarray(inputs["x"], dtype=np.float32)
    wq = np.asarray(inputs["wq"], dtype=np.float32)
    wk = np.asarray(inputs["wk"], dtype=np.float32)
    wv = np.asarray(inputs["wv"], dtype=np.float32)
    wo = np.asarray(inputs["wo"], dtype=np.float32)
    bq = np.asarray(inputs["bq"], dtype=np.float32)
    bk = np.asarray(inputs["bk"], dtype=np.float32)
    bv = np.asarray(inputs["bv"], dtype=np.float32)

    inv = 1.0 / (10000.0 ** (np.arange(0, DH, 2, dtype=np.float64) / DH))
    ang = np.arange(S, dtype=np.float64)[:, None] * inv[None, :]
    sin = np.repeat(np.sin(ang), 2, axis=1).astype(np.float32)  # [S, DH]
    cos = np.repeat(np.cos(ang), 2, axis=1).astype(np.float32)
    # [NQ, DH, SC]: cosq[qi, p, s] = cos[qi*SC+s, p]
    cosq = np.ascontiguousarray(
        cos.reshape(NQ, SC, DH).transpose(0, 2, 1)).astype(bf16)
    sinq = np.ascontiguousarray(
        sin.reshape(NQ, SC, DH).transpose(0, 2, 1)).astype(bf16)

    P = np.zeros((DH, DH), np.float32)
    idx = np.arange(0, DH, 2)
    P[idx, idx + 1] = -1.0    # out[2i]   = -x[2i+1]
    P[idx + 1, idx] = 1.0     # out[2i+1] =  x[2i]
    PT = np.ascontiguousarray(P.T)

    # packed bf16 consts: PT | ones | zeros | tri(keep jj >= i)
    tri = (np.arange(128)[None, :] >= np.arange(128)[:, None])
    cb16 = np.concatenate(
        [PT, np.ones((128, 128), np.float32),
         np.zeros((128, 128), np.float32), tri.astype(np.float32)],
        axis=1).astype(bf16)

    # [NQ, 128, NT, SC]: xTq[qi, p, d, s] = x[b][qi*SC+s, d*128+p]
    xTqb = [np.ascontiguousarray(
        x[b].reshape(NQ, SC, NT, 128).transpose(0, 3, 2, 1)).astype(bf16)
        for b in range(B)]

    in_maps = []
    for core in range(N_CORES):
        b, g = divmod(core, G)
        c0 = g * CW
        # [128, NT, CW]: wqb[p, d, c] = wq[c0+c, d*128+p]
        wqb = np.ascontiguousarray(
            wq[c0:c0 + CW, :].reshape(CW, NT, 128).transpose(2, 1, 0)
        ).astype(bf16)
        wkb = np.ascontiguousarray(
            wk[c0:c0 + CW, :].reshape(CW, NT, 128).transpose(2, 1, 0)
        ).astype(bf16)
        wvb = np.ascontiguousarray(
            wv[c0:c0 + CW, :].reshape(CW, NT, 128).transpose(2, 1, 0)
        ).astype(bf16)
        # [128, HPG, D]: wob[p, h, j] = wo[j, c0+h*128+p]
        wob = np.ascontiguousarray(
            wo[:, c0:c0 + CW].reshape(D, HPG, 128).transpose(2, 1, 0)
        ).astype(bf16)
        # packed f32 consts: bvb (broadcast) | bq columns | bk columns
        cf32 = np.zeros((128, CW + 2 * HPG), np.float32)
        cf32[:, 0:CW] = bv[c0:c0 + CW][None, :]
        cf32[:, CW:CW + HPG] = bq[c0:c0 + CW].reshape(HPG, DH).T
        cf32[:, CW + HPG:] = bk[c0:c0 + CW].reshape(HPG, DH).T
        in_maps.append({
            "xTq": xTqb[b],
            "wqb": wqb,
            "wkb": wkb,
            "wvb": wvb,
            "wob": wob,
            "cf32d": cf32,
            "cb16d": cb16,
            "cosq": cosq,
            "sinq": sinq,
        })
    return in_maps


def _get_nc():
    if "nc" not in _NC_CACHE:
        _NC_CACHE["nc"] = build_attn_nc(iters=1)
    return _NC_CACHE["nc"]


def kernel(**inputs) -> np.ndarray:
    from concourse.bass_utils import run_bass_kernel_spmd

    nc = _get_nc()
    in_maps = host_prep(inputs)
    res = run_bass_kernel_spmd(nc, in_maps, core_ids=list(range(N_CORES)))
    bo = np.asarray(inputs["bo"], dtype=np.float32)
    outp = np.zeros((B, S, D), np.float32)
    for core in range(N_CORES):
        outp[core // G] += np.asarray(res.results[core]["out"],
                                      dtype=np.float32)
    outp += bo[None, None, :]
    return outp
